# revision 20
# baseline (speedup 1.0000x reference)
import numpy as np
import ml_dtypes

B, S, I, H, C = 64, 512, 256, 512, 10
NCORES = 8
BL = B // NCORES
D = 16          # l1 lag behind l0, in steps
CH = 8          # steps per pre-GEMM chunk (= one PSUM bank)
NCHK = S // CH

_cache = {}


def _build_nc():
    from collections import deque

    import concourse.bass as bass
    import concourse.bacc as bacc
    from concourse.bass import mybir

    f32 = mybir.dt.float32
    bf16 = mybir.dt.bfloat16
    Tanh = mybir.ActivationFunctionType.Tanh

    nc = bacc.Bacc("TRN2", target_bir_lowering=False, debug=False, num_devices=NCORES)

    xT_d = nc.dram_tensor("xT", [128, 2, S * BL], bf16, kind="ExternalInput")
    wih0_d = nc.dram_tensor("wih0", [128, 2, 4, 128], bf16, kind="ExternalInput")
    whh0_d = nc.dram_tensor("whh0", [128, 4, 4, 128], bf16, kind="ExternalInput")
    wih1_d = nc.dram_tensor("wih1", [128, 4, 4, 128], bf16, kind="ExternalInput")
    whh1_d = nc.dram_tensor("whh1", [128, 4, 4, 128], bf16, kind="ExternalInput")
    wfc_d = nc.dram_tensor("wfc", [128, 4, C], bf16, kind="ExternalInput")
    b0_d = nc.dram_tensor("b0", [4, 128], bf16, kind="ExternalInput")
    b1_d = nc.dram_tensor("b1", [4, 128], bf16, kind="ExternalInput")
    ones_d = nc.dram_tensor("sel", [4, 4, CH, BL], bf16, kind="ExternalInput")
    bfc_d = nc.dram_tensor("bfc", [C, 1], f32, kind="ExternalInput")
    out_d = nc.dram_tensor("out", [C, BL], f32, kind="ExternalOutput")
    import os
    DBG = os.environ.get("KDBG") == "1"
    if DBG:
        hist_d = nc.dram_tensor("hist", [128, 4, S, BL], bf16, kind="ExternalOutput")


    from contextlib import ExitStack

    with ExitStack() as stack:
        e = stack.enter_context
        block = e(nc.Block())
        dmas = e(nc.semaphore("dmas"))
        h0 = e(nc.semaphore("h0"))
        h1 = e(nc.semaphore("h1"))
        h0y = e(nc.semaphore("h0y"))
        h1y = e(nc.semaphore("h1y"))
        a0 = e(nc.semaphore("a0"))
        a1 = e(nc.semaphore("a1"))
        a0y = e(nc.semaphore("a0y"))
        a1y = e(nc.semaphore("a1y"))
        fcs = e(nc.semaphore("fcs"))
        vs = e(nc.semaphore("vs"))
        xT = e(nc.sbuf_tensor("xT_s", [128, 2, S * BL], bf16))
        wih0 = e(nc.sbuf_tensor("wih0_s", [128, 2, 4, 128], bf16))
        whh0 = e(nc.sbuf_tensor("whh0_s", [128, 4, 4, 128], bf16))
        wih1 = e(nc.sbuf_tensor("wih1_s", [128, 4, 4, 128], bf16))
        whh1 = e(nc.sbuf_tensor("whh1_s", [128, 4, 4, 128], bf16))
        wfc = e(nc.sbuf_tensor("wfc_s", [128, 4, C], bf16))
        b0r = e(nc.sbuf_tensor("b0r", [4, 128], bf16))
        b1r = e(nc.sbuf_tensor("b1r", [4, 128], bf16))
        onesv = e(nc.sbuf_tensor("selv", [4, 4, CH, BL], bf16))
        bfcv = e(nc.sbuf_tensor("bfcv_s", [C, 1], f32))
        hist0 = e(nc.sbuf_tensor("hist0", [128, 4, S, BL], bf16))
        h1r = e(nc.sbuf_tensor("h1r", [128, 4, 2, BL], bf16))
        fco = e(nc.sbuf_tensor("fco", [C, BL], f32))
        ps0a = e(nc.psum_tensor("ps0a", [128, 4, CH, BL], f32))
        ps0b = e(nc.psum_tensor("ps0b", [128, 4, CH, BL], f32))
        ps1a = e(nc.psum_tensor("ps1a", [128, 4, CH, BL], f32))
        ps1b = e(nc.psum_tensor("ps1b", [128, 4, CH, BL], f32))
        ps0 = [ps0a, ps0b]
        ps1 = [ps1a, ps1b]
        fcps = e(nc.psum_tensor("fcps", [128, BL], f32))
        # DMA milestones (each dma_start incs dmas by 16)
        # M1 (=64):  wih0, b0r, onesv, xT[0:64 steps]
        # M2 (=80):  whh0
        # M3 (=128): wih1, b1r, whh1
        # M4 (=144): xT rest
        # M5 (=176): wfc, bfcv
        @block.sync
        def _(sync):
            sync.dma_start(out=wih0[:], in_=wih0_d[:]).then_inc(dmas, 16)
            sync.dma_start(out=b0r[:], in_=b0_d[:]).then_inc(dmas, 16)
            sync.dma_start(out=onesv[:], in_=ones_d[:]).then_inc(dmas, 16)
            sync.dma_start(
                out=xT[:, :, 0 : 64 * BL], in_=xT_d[:, :, 0 : 64 * BL]
            ).then_inc(dmas, 16)
            sync.dma_start(out=whh0[:], in_=whh0_d[:]).then_inc(dmas, 16)
            sync.dma_start(out=wih1[:], in_=wih1_d[:]).then_inc(dmas, 16)
            sync.dma_start(out=b1r[:], in_=b1_d[:]).then_inc(dmas, 16)
            sync.dma_start(out=whh1[:], in_=whh1_d[:]).then_inc(dmas, 16)
            sync.dma_start(
                out=xT[:, :, 64 * BL :], in_=xT_d[:, :, 64 * BL :]
            ).then_inc(dmas, 16)
            sync.dma_start(out=wfc[:], in_=wfc_d[:]).then_inc(dmas, 16)
            sync.dma_start(out=bfcv[:], in_=bfc_d[:]).then_inc(dmas, 16)
            sync.wait_ge(vs, 1)
            sync.dma_start(out=out_d[:], in_=fco[:]).then_inc(dmas, 16)
            if DBG:
                sync.dma_start(out=hist_d[:], in_=hist0[:]).then_inc(dmas, 16)
                sync.wait_ge(dmas, 208)
            else:
                sync.wait_ge(dmas, 192)

        @block.tensor
        def _(te):
            te.wait_ge(dmas, 64)

            def sel0(c):
                def f():
                    if c == 8:
                        te.wait_ge(dmas, 144)  # xT rest
                    if c >= 2:
                        te.wait_ge(a0y, max(0, CH * (c - 1)))
                    te.matmul(
                        ps0[c % 2][:, :, :, :].opt(), b0r[:, :], onesv[:, :, :, :],
                        start=True, stop=False, skip_group_check=True,
                    )
                return f

            def g0(c, kc):
                def f():
                    for jc in range(4):
                        mm = te.matmul(
                            ps0[c % 2][:, jc, :, :].opt(),
                            wih0[:, kc, jc, :],
                            xT[:, kc, c * CH * BL : (c + 1) * CH * BL],
                            start=False, stop=False, skip_group_check=True,
                        )
                        if c == 0 and kc == 1 and jc == 2:
                            mm.then_inc(h0)
                        if c == 0 and kc == 1 and jc == 3:
                            mm.then_inc(h0y)
                return f

            def sel1(c):
                def f():
                    if c == 0:
                        te.wait_ge(dmas, 128)  # wih1, b1r, whh1
                    if c >= 2:
                        te.wait_ge(a1y, max(0, CH * (c - 1)))
                    te.matmul(
                        ps1[c % 2][:, :, :, :].opt(), b1r[:, :], onesv[:, :, :, :],
                        start=True, stop=False, skip_group_check=True,
                    )
                return f

            def g1(c, kc):
                def f():
                    if kc == 0:
                        te.wait_ge(a0y, CH * c + CH)  # h0 history through chunk
                    for jc in range(4):
                        mm = te.matmul(
                            ps1[c % 2][:, jc, :, :].opt(),
                            wih1[:, kc, jc, :],
                            hist0[:, kc, c * CH : (c + 1) * CH, :],
                            start=False, stop=False, skip_group_check=True,
                        )
                        if c == 0 and kc == 3 and jc == 2:
                            mm.then_inc(h1)
                        if c == 0 and kc == 3 and jc == 3:
                            mm.then_inc(h1y)
                return f

            q = deque()
            for c in range(2, NCHK):
                q.append((CH * c - 8, sel0(c)))
                for kc in range(2):
                    q.append((CH * c - 6 + kc, g0(c, kc)))
            q1 = deque()
            for c in range(NCHK):
                q1.append((CH * c + D - 8, sel1(c)))
                for kc in range(4):
                    q1.append((CH * c + D - 6 + kc, g1(c, kc)))

            def pop_ready(t, n):
                done = 0
                while done < n:
                    if q1 and q1[0][0] <= t:
                        q1.popleft()[1]()
                    elif q and q[0][0] <= t:
                        q.popleft()[1]()
                    else:
                        break
                    done += 1

            # prime: l0 chunks 0 and 1
            sel0(0)()
            for kc in range(2):
                g0(0, kc)()
            te.wait_ge(dmas, 80)  # whh0
            sel0(1)()
            for kc in range(2):
                g0(1, kc)()

            for t in range(S + D):
                # l0 scan step t
                if 0 < t < S:
                    te.wait_ge(a0, t)
                    for kc in range(2):
                        for jc in range(4):
                            te.matmul(
                                ps0[(t // CH) % 2][:, jc, t % CH, :],
                                whh0[:, kc, jc, :],
                                hist0[:, kc, t - 1, :],
                                start=False, stop=False,
                                skip_group_check=True,
                            )
                    te.wait_ge(a0y, t)
                    for kc in range(2, 4):
                        for jc in range(4):
                            mm = te.matmul(
                                ps0[(t // CH) % 2][:, jc, t % CH, :],
                                whh0[:, kc, jc, :],
                                hist0[:, kc, t - 1, :],
                                start=False, stop=(kc == 3),
                                skip_group_check=True,
                            )
                            if kc == 3 and jc == 1:
                                mm.then_inc(h0)
                    mm.then_inc(h0y)
                pop_ready(t, 1)
                # l1 scan step s
                s = t - D
                if 0 < s < S:
                    te.wait_ge(a1, s)
                    for kc in range(2):
                        for jc in range(4):
                            te.matmul(
                                ps1[(s // CH) % 2][:, jc, s % CH, :],
                                whh1[:, kc, jc, :],
                                h1r[:, kc, (s - 1) % 2, :],
                                start=False, stop=False,
                                skip_group_check=True,
                            )
                    te.wait_ge(a1y, s)
                    for kc in range(2, 4):
                        for jc in range(4):
                            mm = te.matmul(
                                ps1[(s // CH) % 2][:, jc, s % CH, :],
                                whh1[:, kc, jc, :],
                                h1r[:, kc, (s - 1) % 2, :],
                                start=False, stop=(kc == 3),
                                skip_group_check=True,
                            )
                            if kc == 3 and jc == 1:
                                mm.then_inc(h1)
                    mm.then_inc(h1y)
                pop_ready(t, 1)

            # FC tail
            te.wait_ge(a1y, S)
            te.wait_ge(dmas, 176)
            for kc in range(4):
                mm = te.matmul(
                    fcps[0:C, :], wfc[:, kc, :], h1r[:, kc, (S - 1) % 2, :],
                    start=(kc == 0), stop=(kc == 3),
                )
            mm.then_inc(fcs)

        @block.scalar
        def _(sc):
            for t in range(S + D):
                if t < S:
                    sc.wait_ge(h0, t + 1)
                    sc.activation(
                        hist0[:, 0:2, t, :],
                        ps0[(t // CH) % 2][:, 0:2, t % CH, :],
                        Tanh,
                    ).then_inc(a0)
                    sc.wait_ge(h0y, t + 1)
                    sc.activation(
                        hist0[:, 2:4, t, :],
                        ps0[(t // CH) % 2][:, 2:4, t % CH, :],
                        Tanh,
                    ).then_inc(a0y)
                s = t - D
                if 0 <= s < S:
                    sc.wait_ge(h1, s + 1)
                    sc.activation(
                        h1r[:, 0:2, s % 2, :],
                        ps1[(s // CH) % 2][:, 0:2, s % CH, :],
                        Tanh,
                    ).then_inc(a1)
                    sc.wait_ge(h1y, s + 1)
                    sc.activation(
                        h1r[:, 2:4, s % 2, :],
                        ps1[(s // CH) % 2][:, 2:4, s % CH, :],
                        Tanh,
                    ).then_inc(a1y)

        @block.vector
        def _(ve):
            ve.wait_ge(fcs, 1)
            ve.tensor_scalar_add(fco[:, :], fcps[0:C, :], bfcv[:, 0:1]).then_inc(vs)

    nc.compile()
    return nc


def _make_sel():
    bf = ml_dtypes.bfloat16
    sel = np.zeros((4, 4, CH, BL), dtype=np.float32)
    for jc in range(4):
        sel[jc, jc, :, :] = 1.0
    return sel.astype(bf)


def _prep_inputs(inputs):
    bf = ml_dtypes.bfloat16
    w_ih0 = inputs["w_ih0"]
    w_hh0 = inputs["w_hh0"]
    w_ih1 = inputs["w_ih1"]
    w_hh1 = inputs["w_hh1"]
    w_fc = inputs["w_fc"]

    def lhsT_4(w, n_kc):
        # w: [512, n_kc*128] -> [kp, kc, jc, jp]
        return np.ascontiguousarray(
            w.reshape(4, 128, n_kc, 128).transpose(3, 2, 0, 1)
        ).astype(bf)

    shared = {
        "wih0": lhsT_4(w_ih0, 2),
        "whh0": lhsT_4(w_hh0, 4),
        "wih1": lhsT_4(w_ih1, 4),
        "whh1": lhsT_4(w_hh1, 4),
        "wfc": np.ascontiguousarray(
            w_fc.reshape(C, 4, 128).transpose(2, 1, 0)
        ).astype(bf),
        "b0": np.ascontiguousarray(
            (inputs["b_ih0"] + inputs["b_hh0"]).reshape(4, 128)
        ).astype(bf),
        "b1": np.ascontiguousarray(
            (inputs["b_ih1"] + inputs["b_hh1"]).reshape(4, 128)
        ).astype(bf),
        "sel": _make_sel(),
        "bfc": inputs["b_fc"].reshape(C, 1).astype(np.float32),
    }
    x = inputs["x"]
    in_maps = []
    for c in range(NCORES):
        xs = x[c * BL : (c + 1) * BL]  # [b, t, i]
        xT = (
            np.ascontiguousarray(
                xs.transpose(2, 1, 0).reshape(2, 128, S * BL).transpose(1, 0, 2)
            )
        ).astype(bf)
        m = dict(shared)
        m["xT"] = xT
        in_maps.append(m)
    return in_maps


def kernel(**inputs):
    from concourse import bass_utils

    if "nc" not in _cache:
        _cache["nc"] = _build_nc()
    nc = _cache["nc"]
    in_maps = _prep_inputs(inputs)
    res = bass_utils.run_bass_kernel_spmd(nc, in_maps, core_ids=list(range(NCORES)))
    y = np.concatenate(
        [np.asarray(res.results[c]["out"]).T for c in range(NCORES)], axis=0
    )
    return y.astype(np.float32)


# revision 21
# speedup vs baseline: 1.0209x; 1.0209x over previous
import numpy as np
import ml_dtypes

B, S, I, H, C = 64, 512, 256, 512, 10
NCORES = 8
BL = B // NCORES
D = 16          # l1 lag behind l0, in steps
CH = 8          # steps per pre-GEMM chunk (= one PSUM bank)
NCHK = S // CH

_cache = {}


def _build_nc():
    from collections import deque

    import concourse.bass as bass
    import concourse.bacc as bacc
    from concourse.bass import mybir

    f32 = mybir.dt.float32
    bf16 = mybir.dt.bfloat16
    Tanh = mybir.ActivationFunctionType.Tanh

    nc = bacc.Bacc("TRN2", target_bir_lowering=False, debug=False, num_devices=NCORES)

    xT_d = nc.dram_tensor("xT", [128, 2, S * BL], bf16, kind="ExternalInput")
    wih0_d = nc.dram_tensor("wih0", [128, 2, 4, 128], bf16, kind="ExternalInput")
    whh0_d = nc.dram_tensor("whh0", [128, 4, 4, 128], bf16, kind="ExternalInput")
    wih1_d = nc.dram_tensor("wih1", [128, 4, 4, 128], bf16, kind="ExternalInput")
    whh1_d = nc.dram_tensor("whh1", [128, 4, 4, 128], bf16, kind="ExternalInput")
    wfc_d = nc.dram_tensor("wfc", [128, 4, C], bf16, kind="ExternalInput")
    b0_d = nc.dram_tensor("b0", [4, 128], bf16, kind="ExternalInput")
    b1_d = nc.dram_tensor("b1", [4, 128], bf16, kind="ExternalInput")
    ones_d = nc.dram_tensor("sel", [4, 4, CH, BL], bf16, kind="ExternalInput")
    bfc_d = nc.dram_tensor("bfc", [C, 1], f32, kind="ExternalInput")
    out_d = nc.dram_tensor("out", [C, BL], f32, kind="ExternalOutput")
    import os
    DBG = os.environ.get("KDBG") == "1"
    if DBG:
        hist_d = nc.dram_tensor("hist", [128, 4, S, BL], bf16, kind="ExternalOutput")


    from contextlib import ExitStack

    with ExitStack() as stack:
        e = stack.enter_context
        block = e(nc.Block())
        dmas = e(nc.semaphore("dmas"))
        h0 = e(nc.semaphore("h0"))
        h1 = e(nc.semaphore("h1"))
        h0y = e(nc.semaphore("h0y"))
        h1y = e(nc.semaphore("h1y"))
        a0 = e(nc.semaphore("a0"))
        a1 = e(nc.semaphore("a1"))
        a0y = e(nc.semaphore("a0y"))
        a1y = e(nc.semaphore("a1y"))
        fcs = e(nc.semaphore("fcs"))
        vs = e(nc.semaphore("vs"))
        xT = e(nc.sbuf_tensor("xT_s", [128, 2, S * BL], bf16))
        wih0 = e(nc.sbuf_tensor("wih0_s", [128, 2, 4, 128], bf16))
        whh0 = e(nc.sbuf_tensor("whh0_s", [128, 4, 4, 128], bf16))
        wih1 = e(nc.sbuf_tensor("wih1_s", [128, 4, 4, 128], bf16))
        whh1 = e(nc.sbuf_tensor("whh1_s", [128, 4, 4, 128], bf16))
        wfc = e(nc.sbuf_tensor("wfc_s", [128, 4, C], bf16))
        b0r = e(nc.sbuf_tensor("b0r", [4, 128], bf16))
        b1r = e(nc.sbuf_tensor("b1r", [4, 128], bf16))
        onesv = e(nc.sbuf_tensor("selv", [4, 4, CH, BL], bf16))
        bfcv = e(nc.sbuf_tensor("bfcv_s", [C, 1], f32))
        hist0 = e(nc.sbuf_tensor("hist0", [128, 4, S, BL], bf16))
        h1r = e(nc.sbuf_tensor("h1r", [128, 4, 2, BL], bf16))
        fco = e(nc.sbuf_tensor("fco", [C, BL], f32))
        ps0a = e(nc.psum_tensor("ps0a", [128, 4, CH, BL], f32))
        ps0b = e(nc.psum_tensor("ps0b", [128, 4, CH, BL], f32))
        ps1a = e(nc.psum_tensor("ps1a", [128, 4, CH, BL], f32))
        ps1b = e(nc.psum_tensor("ps1b", [128, 4, CH, BL], f32))
        ps0 = [ps0a, ps0b]
        ps1 = [ps1a, ps1b]
        fcps = e(nc.psum_tensor("fcps", [128, BL], f32))
        # DMA milestones (each dma_start incs dmas by 16)
        # M1 (=64):  wih0, b0r, onesv, xT[0:64 steps]
        # M2 (=80):  whh0
        # M3 (=128): wih1, b1r, whh1
        # M4 (=144): xT rest
        # M5 (=176): wfc, bfcv
        @block.sync
        def _(sync):
            sync.dma_start(out=wih0[:], in_=wih0_d[:]).then_inc(dmas, 16)
            sync.dma_start(out=b0r[:], in_=b0_d[:]).then_inc(dmas, 16)
            sync.dma_start(out=onesv[:], in_=ones_d[:]).then_inc(dmas, 16)
            sync.dma_start(
                out=xT[:, :, 0 : 64 * BL], in_=xT_d[:, :, 0 : 64 * BL]
            ).then_inc(dmas, 16)
            sync.dma_start(out=whh0[:], in_=whh0_d[:]).then_inc(dmas, 16)
            sync.dma_start(out=wih1[:], in_=wih1_d[:]).then_inc(dmas, 16)
            sync.dma_start(out=b1r[:], in_=b1_d[:]).then_inc(dmas, 16)
            sync.dma_start(out=whh1[:], in_=whh1_d[:]).then_inc(dmas, 16)
            sync.dma_start(
                out=xT[:, :, 64 * BL :], in_=xT_d[:, :, 64 * BL :]
            ).then_inc(dmas, 16)
            sync.dma_start(out=wfc[:], in_=wfc_d[:]).then_inc(dmas, 16)
            sync.dma_start(out=bfcv[:], in_=bfc_d[:]).then_inc(dmas, 16)
            sync.wait_ge(vs, 1)
            sync.dma_start(out=out_d[:], in_=fco[:]).then_inc(dmas, 16)
            if DBG:
                sync.dma_start(out=hist_d[:], in_=hist0[:]).then_inc(dmas, 16)
                sync.wait_ge(dmas, 208)
            else:
                sync.wait_ge(dmas, 192)

        @block.tensor
        def _(te):
            te.wait_ge(dmas, 64)

            def sel0(c):
                def f():
                    if c == 8:
                        te.wait_ge(dmas, 144)  # xT rest
                    if c >= 2:
                        te.wait_ge(a0y, max(0, CH * (c - 1)))
                    te.matmul(
                        ps0[c % 2][:, :, :, :].opt(), b0r[:, :], onesv[:, :, :, :],
                        start=True, stop=False, skip_group_check=True,
                    )
                return f

            def g0(c, kc):
                def f():
                    for jc in range(4):
                        mm = te.matmul(
                            ps0[c % 2][:, jc, :, :].opt(),
                            wih0[:, kc, jc, :],
                            xT[:, kc, c * CH * BL : (c + 1) * CH * BL],
                            start=False, stop=False, skip_group_check=True,
                        )
                        if c == 0 and kc == 1 and jc == 2:
                            mm.then_inc(h0)
                        if c == 0 and kc == 1 and jc == 3:
                            mm.then_inc(h0y)
                return f

            def sel1(c):
                def f():
                    if c == 0:
                        te.wait_ge(dmas, 128)  # wih1, b1r, whh1
                    if c >= 2:
                        te.wait_ge(a1y, max(0, CH * (c - 1)))
                    te.matmul(
                        ps1[c % 2][:, :, :, :].opt(), b1r[:, :], onesv[:, :, :, :],
                        start=True, stop=False, skip_group_check=True,
                    )
                return f

            def g1(c, kc):
                def f():
                    if kc == 0:
                        te.wait_ge(a0y, CH * c + CH)  # h0 history through chunk
                    for jc in range(4):
                        mm = te.matmul(
                            ps1[c % 2][:, jc, :, :].opt(),
                            wih1[:, kc, jc, :],
                            hist0[:, kc, c * CH : (c + 1) * CH, :],
                            start=False, stop=False, skip_group_check=True,
                        )
                        if c == 0 and kc == 3 and jc == 2:
                            mm.then_inc(h1)
                        if c == 0 and kc == 3 and jc == 3:
                            mm.then_inc(h1y)
                return f

            q = deque()
            for c in range(2, NCHK):
                q.append((CH * c - 8, sel0(c)))
                for kc in range(2):
                    q.append((CH * c - 6 + kc, g0(c, kc)))
            q1 = deque()
            for c in range(NCHK):
                q1.append((CH * c + D - 8, sel1(c)))
                for kc in range(4):
                    q1.append((CH * c + D - 6 + kc, g1(c, kc)))

            def pop_ready(t, n):
                done = 0
                while done < n:
                    if q1 and q1[0][0] <= t:
                        q1.popleft()[1]()
                    elif q and q[0][0] <= t:
                        q.popleft()[1]()
                    else:
                        break
                    done += 1

            # prime: l0 chunks 0 and 1
            sel0(0)()
            for kc in range(2):
                g0(0, kc)()
            te.wait_ge(dmas, 80)  # whh0
            sel0(1)()
            for kc in range(2):
                g0(1, kc)()

            for t in range(S + D):
                # l0 scan step t
                if 0 < t < S:
                    te.wait_ge(a0, t)
                    for kc in range(2):
                        for jc in range(4):
                            te.matmul(
                                ps0[(t // CH) % 2][:, jc, t % CH, :],
                                whh0[:, kc, jc, :],
                                hist0[:, kc, t - 1, :],
                                start=False, stop=False,
                                skip_group_check=True,
                            )
                    te.wait_ge(a0y, t)
                    for kc in range(2, 4):
                        for jc in range(4):
                            mm = te.matmul(
                                ps0[(t // CH) % 2][:, jc, t % CH, :],
                                whh0[:, kc, jc, :],
                                hist0[:, kc, t - 1, :],
                                start=False, stop=(kc == 3),
                                skip_group_check=True,
                            )
                            if kc == 3 and jc == 1:
                                mm.then_inc(h0)
                    mm.then_inc(h0y)
                pop_ready(t, 2)
                # l1 scan step s
                s = t - D
                if 0 < s < S:
                    te.wait_ge(a1, s)
                    for kc in range(2):
                        for jc in range(4):
                            te.matmul(
                                ps1[(s // CH) % 2][:, jc, s % CH, :],
                                whh1[:, kc, jc, :],
                                h1r[:, kc, (s - 1) % 2, :],
                                start=False, stop=False,
                                skip_group_check=True,
                            )
                    te.wait_ge(a1y, s)
                    for kc in range(2, 4):
                        for jc in range(4):
                            mm = te.matmul(
                                ps1[(s // CH) % 2][:, jc, s % CH, :],
                                whh1[:, kc, jc, :],
                                h1r[:, kc, (s - 1) % 2, :],
                                start=False, stop=(kc == 3),
                                skip_group_check=True,
                            )
                            if kc == 3 and jc == 1:
                                mm.then_inc(h1)
                    mm.then_inc(h1y)
                pop_ready(t, 2)

            # FC tail
            te.wait_ge(a1y, S)
            te.wait_ge(dmas, 176)
            for kc in range(4):
                mm = te.matmul(
                    fcps[0:C, :], wfc[:, kc, :], h1r[:, kc, (S - 1) % 2, :],
                    start=(kc == 0), stop=(kc == 3),
                )
            mm.then_inc(fcs)

        @block.scalar
        def _(sc):
            for t in range(S + D):
                if t < S:
                    sc.wait_ge(h0, t + 1)
                    sc.activation(
                        hist0[:, 0:2, t, :],
                        ps0[(t // CH) % 2][:, 0:2, t % CH, :],
                        Tanh,
                    ).then_inc(a0)
                    sc.wait_ge(h0y, t + 1)
                    sc.activation(
                        hist0[:, 2:4, t, :],
                        ps0[(t // CH) % 2][:, 2:4, t % CH, :],
                        Tanh,
                    ).then_inc(a0y)
                s = t - D
                if 0 <= s < S:
                    sc.wait_ge(h1, s + 1)
                    sc.activation(
                        h1r[:, 0:2, s % 2, :],
                        ps1[(s // CH) % 2][:, 0:2, s % CH, :],
                        Tanh,
                    ).then_inc(a1)
                    sc.wait_ge(h1y, s + 1)
                    sc.activation(
                        h1r[:, 2:4, s % 2, :],
                        ps1[(s // CH) % 2][:, 2:4, s % CH, :],
                        Tanh,
                    ).then_inc(a1y)

        @block.vector
        def _(ve):
            ve.wait_ge(fcs, 1)
            ve.tensor_scalar_add(fco[:, :], fcps[0:C, :], bfcv[:, 0:1]).then_inc(vs)

    nc.compile()
    return nc


def _make_sel():
    bf = ml_dtypes.bfloat16
    sel = np.zeros((4, 4, CH, BL), dtype=np.float32)
    for jc in range(4):
        sel[jc, jc, :, :] = 1.0
    return sel.astype(bf)


def _prep_inputs(inputs):
    bf = ml_dtypes.bfloat16
    w_ih0 = inputs["w_ih0"]
    w_hh0 = inputs["w_hh0"]
    w_ih1 = inputs["w_ih1"]
    w_hh1 = inputs["w_hh1"]
    w_fc = inputs["w_fc"]

    def lhsT_4(w, n_kc):
        # w: [512, n_kc*128] -> [kp, kc, jc, jp]
        return np.ascontiguousarray(
            w.reshape(4, 128, n_kc, 128).transpose(3, 2, 0, 1)
        ).astype(bf)

    shared = {
        "wih0": lhsT_4(w_ih0, 2),
        "whh0": lhsT_4(w_hh0, 4),
        "wih1": lhsT_4(w_ih1, 4),
        "whh1": lhsT_4(w_hh1, 4),
        "wfc": np.ascontiguousarray(
            w_fc.reshape(C, 4, 128).transpose(2, 1, 0)
        ).astype(bf),
        "b0": np.ascontiguousarray(
            (inputs["b_ih0"] + inputs["b_hh0"]).reshape(4, 128)
        ).astype(bf),
        "b1": np.ascontiguousarray(
            (inputs["b_ih1"] + inputs["b_hh1"]).reshape(4, 128)
        ).astype(bf),
        "sel": _make_sel(),
        "bfc": inputs["b_fc"].reshape(C, 1).astype(np.float32),
    }
    x = inputs["x"]
    in_maps = []
    for c in range(NCORES):
        xs = x[c * BL : (c + 1) * BL]  # [b, t, i]
        xT = (
            np.ascontiguousarray(
                xs.transpose(2, 1, 0).reshape(2, 128, S * BL).transpose(1, 0, 2)
            )
        ).astype(bf)
        m = dict(shared)
        m["xT"] = xT
        in_maps.append(m)
    return in_maps


def kernel(**inputs):
    from concourse import bass_utils

    if "nc" not in _cache:
        _cache["nc"] = _build_nc()
    nc = _cache["nc"]
    in_maps = _prep_inputs(inputs)
    res = bass_utils.run_bass_kernel_spmd(nc, in_maps, core_ids=list(range(NCORES)))
    y = np.concatenate(
        [np.asarray(res.results[c]["out"]).T for c in range(NCORES)], axis=0
    )
    return y.astype(np.float32)
